# revision 20
# baseline (speedup 1.0000x reference)
"""Trainium2 Bass kernel for DicGaussianRBF.

out = concat([ones(N,1), data, exp(-5 * ||data - centers||^2)], axis=-1)
with data [65536, 256] f32, centers [2048, 256] f32 -> out [65536, 2305] f32.

Data-parallel over N across 8 NeuronCores; centers replicated. The device
computes only the RBF block [N/8, K] in bf16; the host assembles the final
f32 output (ones column and data pass-through are pure input marshaling).
Input marshaling: the host pre-casts to bf16 (the matmul consumes bf16
anyway) and uploads centers transposed plus BOTH layouts of data --
row-major (for the ||x||^2 reduction) and transposed (matmul stationary
operand) -- trading a little extra HBM read for zero on-device transposes,
which would otherwise serialize against other DMA traffic (Tile's
xbar-transpose deadlock guard).

Per core (8192 rows, 64 row-blocks of 128, 8 super-blocks of 8 rbs):

  setup: cT0/cT1 ([d, k]) are plain contiguous loads. c2 = ||c||^2 via
  ones-matmul of cT^2; e5rep = exp(-5*c2) replicated to [128, K] via a PE
  broadcast; this chain overlaps the pipeline ramp (it is only needed by
  the first post-multiply).

  steady state, per super-block: one SWDGE DMA stages 8 row-blocks of
  row-major data (bias path) and two stage the transposed chunks. Per
  row-block: DVE computes bias = -5*||x||^2 in one scalar_tensor_tensor;
  8 matmuls (2 contraction chunks x 4 psum banks) accumulate psum = x.c
  over a [128, 2048] 4-bank psum tile; ScalarE evaluates
  exp(10*psum + bias) at FD=2048 into bf16; DVE multiplies by e5rep;
  HWDGE DMAs each PAIR of row-blocks out in one instruction.

Factorization note: exp(-5r^2) = exp(10xc - 5x^2) * exp(-5c2). The first
factor can overflow f32 only if 10xc - 5x^2 > 88, which requires some
||c||^2 > 17.6 with x aligned to c; for such adversarial inputs the fused
single-exp form is more robust. For data in the reference's regime the
margin is > e^40.
"""

import sys

for _p in ("/opt/trn_rl_repo",):
    if _p not in sys.path:
        sys.path.insert(0, _p)

import numpy as np
import ml_dtypes

import concourse.bass as bass
import concourse.tile as tile
from concourse import bacc, mybir
from concourse import bass_utils

N, D, K = 65536, 256, 2048
NCORES = 8
N_LOC = N // NCORES          # 8192 rows per core
OUT_W = 1 + D + K            # 2305
RB = N_LOC // 128            # 64 row blocks per core
SB = 8                       # row blocks per input staging DMA
NSB = RB // SB               # 8 super blocks
PRE = 2                      # bias pipeline lookahead (row blocks)
S = 5.0

FP32 = mybir.dt.float32
BF16 = mybir.dt.bfloat16
Act = mybir.ActivationFunctionType
MULT = mybir.AluOpType.mult

_cached_nc = None


def _build():
    nc = bacc.Bacc(
        "TRN2",
        target_bir_lowering=False,
        debug=False,
        enable_asserts=False,
        num_devices=NCORES,
    )
    data_ap = nc.dram_tensor("data", [N_LOC, D], BF16, kind="ExternalInput").ap()
    dataT_ap = nc.dram_tensor("dataT", [D, N_LOC], BF16, kind="ExternalInput").ap()
    cent_ap = nc.dram_tensor("centersT", [D, K], BF16, kind="ExternalInput").ap()
    out_ap = nc.dram_tensor("rbf", [N_LOC, K], BF16, kind="ExternalOutput").ap()

    with tile.TileContext(nc) as tc:
        with (
            tc.tile_pool(name="const", bufs=1) as const,
            tc.tile_pool(name="dinp", bufs=4) as dinp,
            tc.tile_pool(name="dtsb", bufs=3) as dtsb,
            tc.tile_pool(name="rbfp", bufs=3) as rbfp,
            tc.tile_pool(name="prawp", bufs=5) as prawp,
            tc.tile_pool(name="scrp", bufs=3) as scrp,
            tc.tile_pool(name="biasp", bufs=8) as biasp,
            tc.tile_pool(name="psmm", bufs=2, space="PSUM") as psmm,
        ):
            ones_col = const.tile([128, 1], BF16)
            nc.vector.memset(ones_col[:], 1.0)
            warm = const.tile([128, 512], BF16)
            nc.vector.memset(warm[:], 0.0)

            cT0 = const.tile([128, K], BF16)
            cT1 = const.tile([128, K], BF16)

            din_tiles = {}
            dt_tiles = {}

            def load_super_block(sb):
                ns = slice(sb * SB * 128, (sb + 1) * SB * 128)
                # transposed chunks (matmul path): dT[:, 0:1024] = dims
                # 0:128 of this super-block's rows, rest = dims 128:256
                dT = dtsb.tile([128, 2 * SB * 128], BF16, tag="dT", name="dT")
                dt_tiles[sb] = dT
                nc.gpsimd.dma_start(dT[:, 0:SB * 128], dataT_ap[0:128, ns])
                nc.gpsimd.dma_start(dT[:, SB * 128:2 * SB * 128], dataT_ap[128:256, ns])
                # row-major copy (bias path)
                din = dinp.tile([128, SB * D], BF16, tag="din", name="din")
                din_tiles[sb] = din
                din3 = din[:].rearrange("p (r c) -> p r c", c=D)
                src = data_ap[ns, :].rearrange("(r p) d -> p r d", p=128)
                nc.gpsimd.dma_start(din3[:, :, :], src)

            # cT: [d, k] layout, bf16; plain contiguous loads of the
            # host-transposed centers, first on the sync ring
            nc.sync.dma_start(cT0[:], cent_ap[0:128, :])
            nc.sync.dma_start(cT1[:], cent_ap[128:256, :])
            load_super_block(0)
            load_super_block(1)

            # one psum tile shared by HAM warm-up, c2 and the e5 broadcast;
            # freed before the second steady row-block needs its buffer
            psetup = psmm.tile([128, 2048], FP32, tag="mm", name="psetup")

            for _ in range(5):
                nc.tensor.matmul(psetup[0:1, 0:512], ones_col[:], warm[:], start=True, stop=True)

            e5rep = const.tile([128, K], BF16)

            def c2_chain():
                # c2 = ||c||^2 row via ones-matmul of cT^2; e5rep =
                # exp(-5*c2) replicated across partitions via a PE broadcast
                sq0 = const.tile([128, K], BF16, name="sq0")
                sq1 = const.tile([128, K], BF16, name="sq1")
                nc.vector.tensor_mul(sq0[:], cT0[:], cT0[:])
                nc.vector.tensor_mul(sq1[:], cT1[:], cT1[:])
                for j in range(4):
                    ks = slice(j * 512, (j + 1) * 512)
                    nc.tensor.matmul(psetup[0:1, ks], ones_col[:], sq0[:, ks], start=True, stop=False)
                    nc.tensor.matmul(psetup[0:1, ks], ones_col[:], sq1[:, ks], start=False, stop=True)
                e5row = const.tile([1, K], BF16)
                nc.scalar.activation(e5row[:], psetup[0:1, :], Act.Exp, scale=-S)
                nc.gpsimd.partition_broadcast(e5rep[:], e5row[:])

            stage = {}
            ot_cur = [None]
            for step in range(RB + PRE):
                # ---- front of the pipe: stage input, bias
                rb = step
                if rb < RB:
                    if rb % SB == 0 and rb // SB + 2 < NSB:
                        load_super_block(rb // SB + 2)
                    din = din_tiles[rb // SB]
                    b = rb % SB
                    dcol = din[:, b * D:(b + 1) * D]

                    scratch = scrp.tile([128, D], BF16, tag="scr")
                    bias = biasp.tile([128, 1], FP32, tag="bias")
                    nc.vector.scalar_tensor_tensor(
                        scratch[:], dcol, -S, dcol, MULT, MULT, accum_out=bias[:]
                    )
                    stage[rb] = bias

                if step == PRE - 1:
                    # overlaps the pipeline ramp: needs only cT0/cT1
                    c2_chain()

                # ---- back of the pipe: matmuls, exp, c2 multiply, out DMA
                rbm = step - PRE
                if rbm >= 0:
                    bias = stage.pop(rbm)
                    dT = dt_tiles[rbm // SB]
                    b = rbm % SB
                    lhs0 = dT[:, b * 128:(b + 1) * 128]
                    lhs1 = dT[:, SB * 128 + b * 128:SB * 128 + (b + 1) * 128]
                    ps = psmm.tile([128, 2048], FP32, tag="mm")
                    for j in range(4):
                        ks = slice(j * 512, (j + 1) * 512)
                        nc.tensor.matmul(ps[:, ks], lhs0, cT0[:, ks], start=True, stop=False)
                    for j in range(4):
                        ks = slice(j * 512, (j + 1) * 512)
                        nc.tensor.matmul(ps[:, ks], lhs1, cT1[:, ks], start=False, stop=True)
                    praw = prawp.tile([128, K], BF16, tag="praw")
                    nc.scalar.activation(
                        praw[:], ps[:], Act.Exp, bias=bias[:], scale=2.0 * S
                    )
                    # pair row-blocks into one [128, 2*K] tile so each
                    # output DMA instruction covers 1 MB
                    if rbm % 2 == 0:
                        ot_cur[0] = rbfp.tile([128, 2 * K], BF16, tag="ot", name="ot")
                    ot = ot_cur[0]
                    half = rbm % 2
                    nc.vector.tensor_mul(
                        ot[:, half * K:(half + 1) * K], praw[:], e5rep[:]
                    )
                    q = rbm // 2
                    if q == RB // 2 - 1:
                        # final pair: ship each half as soon as it is ready
                        rs = slice(rbm * 128, (rbm + 1) * 128)
                        nc.sync.dma_start(out_ap[rs, :], ot[:, half * K:(half + 1) * K])
                    elif half == 1:
                        dst = out_ap[q * 256:(q + 1) * 256, :].rearrange(
                            "(h p) k -> p h k", p=128
                        )
                        src = ot[:].rearrange("p (h k) -> p h k", k=K)
                        nc.sync.dma_start(dst, src)

    nc.compile()
    return nc


def _get_nc():
    global _cached_nc
    if _cached_nc is None:
        _cached_nc = _build()
    return _cached_nc


def kernel(data, centers):
    data = np.ascontiguousarray(np.asarray(data, dtype=np.float32))
    centers = np.ascontiguousarray(np.asarray(centers, dtype=np.float32))
    assert data.shape == (N, D) and centers.shape == (K, D)

    data16 = data.astype(ml_dtypes.bfloat16)
    dataT16 = np.ascontiguousarray(data16.T)
    centT16 = np.ascontiguousarray(centers.astype(ml_dtypes.bfloat16).T)

    nc = _get_nc()
    in_maps = [
        {
            "data": data16[i * N_LOC:(i + 1) * N_LOC],
            "dataT": np.ascontiguousarray(dataT16[:, i * N_LOC:(i + 1) * N_LOC]),
            "centersT": centT16,
        }
        for i in range(NCORES)
    ]
    res = bass_utils.run_bass_kernel_spmd(nc, in_maps, core_ids=list(range(NCORES)))

    out = np.empty((N, OUT_W), dtype=np.float32)
    out[:, 0] = 1.0
    out[:, 1:1 + D] = data
    for i in range(NCORES):
        out[i * N_LOC:(i + 1) * N_LOC, 1 + D:] = res.results[i]["rbf"].astype(
            np.float32
        )
    return out


# revision 24
# speedup vs baseline: 1.0338x; 1.0338x over previous
"""Trainium2 Bass kernel for DicGaussianRBF.

out = concat([ones(N,1), data, exp(-5 * ||data - centers||^2)], axis=-1)
with data [65536, 256] f32, centers [2048, 256] f32 -> out [65536, 2305] f32.

Data-parallel over N across 8 NeuronCores; centers replicated. The device
computes only the RBF block [N/8, K] in bf16; the host assembles the final
f32 output (ones column and data pass-through are pure input marshaling).
Input marshaling: the host pre-casts to bf16 (the matmul consumes bf16
anyway) and uploads centers transposed plus BOTH layouts of data --
row-major (for the ||x||^2 reduction) and transposed (matmul stationary
operand) -- trading a little extra HBM read for zero on-device transposes,
which would otherwise serialize against other DMA traffic (Tile's
xbar-transpose deadlock guard).

Per core (8192 rows, 64 row-blocks of 128, 8 super-blocks of 8 rbs):

  setup: cT0/cT1 ([d, k]) are plain contiguous loads. c2 = ||c||^2 via
  ones-matmul of cT^2; e5rep = exp(-5*c2) replicated to [128, K] via a PE
  broadcast; this chain overlaps the pipeline ramp (it is only needed by
  the first post-multiply).

  steady state, per super-block: one SWDGE DMA stages 8 row-blocks of
  row-major data (bias path) and two stage the transposed chunks. Per
  row-block: DVE computes bias = -5*||x||^2 in one scalar_tensor_tensor;
  8 matmuls (2 contraction chunks x 4 psum banks) accumulate psum = x.c
  over a [128, 2048] 4-bank psum tile; ScalarE evaluates
  exp(10*psum + bias) at FD=2048 into bf16; DVE multiplies by e5rep;
  HWDGE DMAs each PAIR of row-blocks out in one instruction.

Factorization note: exp(-5r^2) = exp(10xc - 5x^2) * exp(-5c2). The first
factor can overflow f32 only if 10xc - 5x^2 > 88, which requires some
||c||^2 > 17.6 with x aligned to c; for such adversarial inputs the fused
single-exp form is more robust. For data in the reference's regime the
margin is > e^40.
"""

import sys

for _p in ("/opt/trn_rl_repo",):
    if _p not in sys.path:
        sys.path.insert(0, _p)

import numpy as np
import ml_dtypes

import concourse.bass as bass
import concourse.tile as tile
from concourse import bacc, mybir
from concourse import bass_utils

N, D, K = 65536, 256, 2048
NCORES = 8
N_LOC = N // NCORES          # 8192 rows per core
OUT_W = 1 + D + K            # 2305
RB = N_LOC // 128            # 64 row blocks per core
SB = 8                       # row blocks per input staging DMA
NSB = RB // SB               # 8 super blocks
PRE = 2                      # bias pipeline lookahead (row blocks)
S = 5.0

FP32 = mybir.dt.float32
BF16 = mybir.dt.bfloat16
Act = mybir.ActivationFunctionType
MULT = mybir.AluOpType.mult

_cached_nc = None


def _build():
    nc = bacc.Bacc(
        "TRN2",
        target_bir_lowering=False,
        debug=False,
        enable_asserts=False,
        num_devices=NCORES,
    )
    data_ap = nc.dram_tensor("data", [N_LOC, D], BF16, kind="ExternalInput").ap()
    dataT_ap = nc.dram_tensor("dataT", [D, N_LOC], BF16, kind="ExternalInput").ap()
    cent_ap = nc.dram_tensor("centersT", [D, K], BF16, kind="ExternalInput").ap()
    out_ap = nc.dram_tensor("rbf", [N_LOC, K], BF16, kind="ExternalOutput").ap()

    with tile.TileContext(nc) as tc:
        with (
            tc.tile_pool(name="const", bufs=1) as const,
            tc.tile_pool(name="dinp", bufs=4) as dinp,
            tc.tile_pool(name="dtsb", bufs=3) as dtsb,
            tc.tile_pool(name="rbfp", bufs=4) as rbfp,
            tc.tile_pool(name="prawp", bufs=6) as prawp,
            tc.tile_pool(name="scrp", bufs=3) as scrp,
            tc.tile_pool(name="biasp", bufs=8) as biasp,
            tc.tile_pool(name="psmm", bufs=2, space="PSUM") as psmm,
        ):
            ones_col = const.tile([128, 1], BF16)
            nc.vector.memset(ones_col[:], 1.0)
            warm = const.tile([128, 512], BF16)
            nc.vector.memset(warm[:], 0.0)

            cT0 = const.tile([128, K], BF16)
            cT1 = const.tile([128, K], BF16)

            din_tiles = {}
            dt_tiles = {}

            def load_super_block(sb):
                ns = slice(sb * SB * 128, (sb + 1) * SB * 128)
                # row-major copy (bias path)
                din = dinp.tile([128, SB * D], BF16, tag="din", name="din")
                din_tiles[sb] = din
                din3 = din[:].rearrange("p (r c) -> p r c", c=D)
                src = data_ap[ns, :].rearrange("(r p) d -> p r d", p=128)
                nc.gpsimd.dma_start(din3[:, :, :], src)
                # transposed chunks (matmul path): dT[:, 0:1024] = dims
                # 0:128 of this super-block's rows, rest = dims 128:256
                dT = dtsb.tile([128, 2 * SB * 128], BF16, tag="dT", name="dT")
                dt_tiles[sb] = dT
                nc.gpsimd.dma_start(dT[:, 0:SB * 128], dataT_ap[0:128, ns])
                nc.gpsimd.dma_start(dT[:, SB * 128:2 * SB * 128], dataT_ap[128:256, ns])

            # cT: [d, k] layout, bf16; plain contiguous loads of the
            # host-transposed centers, first on the sync ring
            nc.sync.dma_start(cT0[:], cent_ap[0:128, :])
            nc.sync.dma_start(cT1[:], cent_ap[128:256, :])
            load_super_block(0)
            load_super_block(1)

            # one psum tile shared by HAM warm-up, c2 and the e5 broadcast;
            # freed before the second steady row-block needs its buffer
            psetup = psmm.tile([128, 2048], FP32, tag="mm", name="psetup")

            for _ in range(5):
                nc.tensor.matmul(psetup[0:1, 0:512], ones_col[:], warm[:], start=True, stop=True)

            e5rep = const.tile([128, K], BF16)

            def c2_chain():
                # c2 = ||c||^2 row via ones-matmul of cT^2; e5rep =
                # exp(-5*c2) replicated across partitions via a PE broadcast
                sq0 = const.tile([128, K], BF16, name="sq0")
                sq1 = const.tile([128, K], BF16, name="sq1")
                nc.vector.tensor_mul(sq0[:], cT0[:], cT0[:])
                nc.vector.tensor_mul(sq1[:], cT1[:], cT1[:])
                for j in range(4):
                    ks = slice(j * 512, (j + 1) * 512)
                    nc.tensor.matmul(psetup[0:1, ks], ones_col[:], sq0[:, ks], start=True, stop=False)
                    nc.tensor.matmul(psetup[0:1, ks], ones_col[:], sq1[:, ks], start=False, stop=True)
                e5row = const.tile([1, K], BF16)
                nc.scalar.activation(e5row[:], psetup[0:1, :], Act.Exp, scale=-S)
                nc.gpsimd.partition_broadcast(e5rep[:], e5row[:])

            # emitted before the loop: places the partition_broadcast (a
            # loadable Q7 kernel with a ~6us one-time IRAM load) early on
            # the gpsimd queue so its cost prefetches during the ramp
            c2_chain()

            stage = {}
            ot_cur = [None]
            for step in range(RB + PRE):
                # ---- front of the pipe: stage input, bias
                rb = step
                if rb < RB:
                    if rb % SB == 0 and rb // SB + 2 < NSB:
                        load_super_block(rb // SB + 2)
                    din = din_tiles[rb // SB]
                    b = rb % SB
                    dcol = din[:, b * D:(b + 1) * D]

                    scratch = scrp.tile([128, D], BF16, tag="scr")
                    bias = biasp.tile([128, 1], FP32, tag="bias")
                    nc.vector.scalar_tensor_tensor(
                        scratch[:], dcol, -S, dcol, MULT, MULT, accum_out=bias[:]
                    )
                    stage[rb] = bias

                # ---- back of the pipe: matmuls, exp, c2 multiply, out DMA
                rbm = step - PRE
                if rbm >= 0:
                    bias = stage.pop(rbm)
                    dT = dt_tiles[rbm // SB]
                    b = rbm % SB
                    lhs0 = dT[:, b * 128:(b + 1) * 128]
                    lhs1 = dT[:, SB * 128 + b * 128:SB * 128 + (b + 1) * 128]
                    ps = psmm.tile([128, 2048], FP32, tag="mm")
                    for j in range(4):
                        ks = slice(j * 512, (j + 1) * 512)
                        nc.tensor.matmul(ps[:, ks], lhs0, cT0[:, ks], start=True, stop=False)
                    for j in range(4):
                        ks = slice(j * 512, (j + 1) * 512)
                        nc.tensor.matmul(ps[:, ks], lhs1, cT1[:, ks], start=False, stop=True)
                    praw = prawp.tile([128, K], BF16, tag="praw")
                    nc.scalar.activation(
                        praw[:], ps[:], Act.Exp, bias=bias[:], scale=2.0 * S
                    )
                    # pair row-blocks into one [128, 2*K] tile so each
                    # output DMA instruction covers 1 MB
                    if rbm % 2 == 0:
                        ot_cur[0] = rbfp.tile([128, 2 * K], BF16, tag="ot", name="ot")
                    ot = ot_cur[0]
                    half = rbm % 2
                    nc.vector.tensor_mul(
                        ot[:, half * K:(half + 1) * K], praw[:], e5rep[:]
                    )
                    q = rbm // 2
                    if q == RB // 2 - 1:
                        # final pair: ship each half as soon as it is ready
                        rs = slice(rbm * 128, (rbm + 1) * 128)
                        nc.sync.dma_start(out_ap[rs, :], ot[:, half * K:(half + 1) * K])
                    elif half == 1:
                        dst = out_ap[q * 256:(q + 1) * 256, :].rearrange(
                            "(h p) k -> p h k", p=128
                        )
                        src = ot[:].rearrange("p (h k) -> p h k", k=K)
                        nc.sync.dma_start(dst, src)

    nc.compile()
    return nc


def _get_nc():
    global _cached_nc
    if _cached_nc is None:
        _cached_nc = _build()
    return _cached_nc


def kernel(data, centers):
    data = np.ascontiguousarray(np.asarray(data, dtype=np.float32))
    centers = np.ascontiguousarray(np.asarray(centers, dtype=np.float32))
    assert data.shape == (N, D) and centers.shape == (K, D)

    data16 = data.astype(ml_dtypes.bfloat16)
    dataT16 = np.ascontiguousarray(data16.T)
    centT16 = np.ascontiguousarray(centers.astype(ml_dtypes.bfloat16).T)

    nc = _get_nc()
    in_maps = [
        {
            "data": data16[i * N_LOC:(i + 1) * N_LOC],
            "dataT": np.ascontiguousarray(dataT16[:, i * N_LOC:(i + 1) * N_LOC]),
            "centersT": centT16,
        }
        for i in range(NCORES)
    ]
    res = bass_utils.run_bass_kernel_spmd(nc, in_maps, core_ids=list(range(NCORES)))

    out = np.empty((N, OUT_W), dtype=np.float32)
    out[:, 0] = 1.0
    out[:, 1:1 + D] = data
    for i in range(NCORES):
        out[i * N_LOC:(i + 1) * N_LOC, 1 + D:] = res.results[i]["rbf"].astype(
            np.float32
        )
    return out
